# revision 6
# baseline (speedup 1.0000x reference)
"""Trainium2 Bass kernel for ConvTemporalGraphical (gnn_message_passing).

Reference computation (fp32):
    y   = einsum('nctv,oc->notv', x, W) + b        # 1x1 conv channel mix
    out = einsum('nkctv,kvw->nctw', y.reshape(n,K,C,t,v), A)

Shapes: x [16,128,256,64] f32, A [3,64,64], W [384,128], b [384].

Strategy (8 NeuronCores, data-parallel over N, 2 samples per core):
  The two contractions are reordered as
      Z_k[ci,t,w] = sum_v x[ci,t,v] * A[k,v,w]          (graph mixing first)
      out[c,t,w]  = sum_k sum_ci W[(k,c),ci] * Z_k[ci,t,w] + bias2[c,w]
  with bias2[c,w] = sum_{k,v} b[(k,c)] A[k,v,w] (host-precomputed).

  On-device per (n, 32-t chunk):
    1. DMA x tile [ci=128, 32*64] (contiguous 8KB/partition).
    2. PE-transpose per 2-t pair: [ci,128] -> xt [(t0 v|t1 v), ci], fp32r.
       Four transposes share one PSUM tile so the drain is one [128,512] copy.
    3. Step A matmul (fp32r, FD=384): lhsT=xt pair, rhs=MA where MA [128,384]
       is block-diag([Acat, Acat]), Acat[v,(k w)]=A[k,v,w]. The zero blocks
       keep the two t's of a pair independent while using all 128 partitions.
       Two pair-outputs share one PSUM tile; Z accumulates into a
       [ci, 32, 3, 64] SBUF buffer.
    4. Step B matmul (fp32r, FD=512): per 8-t group, accumulate over k in
       PSUM: lhsT=Wt[:,k,:] ([ci,c]), rhs=Z[:, g8, k, :] (strided).
    5. Drain with fused bias add (DVE) -> out tile [c, 32, 64] -> DMA out.

  fp32r (tf32-like) matmuls run at 1 cycle/row for FD>=256 with ~1.6e-4
  relative rounding error; the PE rounds operands internally so DMA/copy
  producers don't need explicit rounding passes. Transposing in fp32r is
  exact w.r.t. the final result: it pre-rounds x exactly as step A would.

kernel(**inputs) shards on host, runs the SPMD program on cores 0-7, and
concatenates the per-core outputs.
"""

import numpy as np

import concourse.bass as bass
import concourse.mybir as mybir
from concourse import bacc
from concourse.bass_utils import run_bass_kernel_spmd
from concourse.tile import TileContext

F32 = mybir.dt.float32
F32R = mybir.dt.float32r

N, C_IN, C_OUT, K, T, V = 16, 128, 128, 3, 256, 64
N_CORES = 8
N_PER_CORE = N // N_CORES  # 2
TC = 32                    # t-chunk size
N_CHUNKS = T // TC         # 8
QG = TC // 8               # 4 quad-groups (8 t's = 4 pairs) per chunk


def build(reps: int = 1):
    nc = bacc.Bacc(
        "TRN2", target_bir_lowering=False, debug=False, num_devices=N_CORES
    )
    xs = nc.dram_tensor("xs", [N_PER_CORE, C_IN, T, V], F32, kind="ExternalInput")
    wt = nc.dram_tensor("wt", [C_IN, K, C_OUT], F32, kind="ExternalInput")
    ma = nc.dram_tensor("ma", [128, 2, K, V], F32, kind="ExternalInput")
    bias2r = nc.dram_tensor("bias2r", [C_OUT, 8, V], F32, kind="ExternalInput")
    ident = nc.dram_tensor("ident", [128, 128], F32, kind="ExternalInput")
    out = nc.dram_tensor(
        "out", [N_PER_CORE, C_OUT, T, V], F32, kind="ExternalOutput"
    )

    with TileContext(nc) as tc:
        with (
            tc.tile_pool(name="const", bufs=1) as cpool,
            tc.tile_pool(name="xin", bufs=3) as xpool,
            tc.tile_pool(name="xt", bufs=3) as xtpool,
            tc.tile_pool(name="z", bufs=2) as zpool,
            tc.tile_pool(name="o", bufs=3) as opool,
            tc.tile_pool(name="ps_xt", bufs=2, space="PSUM") as ps_xt,
            tc.tile_pool(name="ps_z", bufs=2, space="PSUM") as ps_z,
            tc.tile_pool(name="ps_o", bufs=2, space="PSUM") as ps_o,
        ):
            ident_sb = cpool.tile([128, 128], F32R, tag="ident")
            nc.sync.dma_start(out=ident_sb[:], in_=ident[:].bitcast(F32R))
            wt_sb = cpool.tile([C_IN, K, C_OUT], F32R, tag="wt")
            nc.sync.dma_start(out=wt_sb[:], in_=wt[:].bitcast(F32R))
            ma_sb = cpool.tile([128, 2, K, V], F32R, tag="ma")
            nc.sync.dma_start(out=ma_sb[:], in_=ma[:].bitcast(F32R))
            bias_sb = cpool.tile([C_OUT, 8, V], F32, tag="bias")
            nc.sync.dma_start(out=bias_sb[:], in_=bias2r[:])

            # Software-pipelined emission: PE's stream is in-order, so a
            # matmul that depends on a same-stage drain stalls the PE for
            # the full DVE/ACT round trip. Emit transposes of group i,
            # step A of group i-1, and step B of group i-2 so every PE op's
            # producer drain has a full group-time to land.
            for _ in range(reps):
                groups = [
                    (n, c, q)
                    for n in range(N_PER_CORE)
                    for c in range(N_CHUNKS)
                    for q in range(QG)
                ]
                st = {}  # (n, c) -> chunk state

                def chunk_state(n, c):
                    if (n, c) not in st:
                        x_sb = xpool.tile([C_IN, TC * V], F32R, tag="x")
                        nc.sync.dma_start(
                            out=x_sb[:],
                            in_=xs[n, :, c * TC : (c + 1) * TC, :].bitcast(F32R),
                        )
                        st[(n, c)] = {
                            "x": x_sb,
                            "z": zpool.tile(
                                [C_IN, TC, K, V], F32R, tag="z", name="z_sb"
                            ),
                            "o": opool.tile(
                                [C_OUT, TC, V], F32, tag="o", name="o_sb"
                            ),
                            "xt": {},
                        }
                    return st[(n, c)]

                def stage_tp(n, c, q):
                    s = chunk_state(n, c)
                    # 4 transposes -> one PSUM bank as ONE accumulation group
                    # (start clears the bank, so only the first sets it)
                    xt_ps = ps_xt.tile([128, 4, 128], F32R, tag="xtp")
                    for j in range(4):
                        jp = 4 * q + j
                        nc.tensor.matmul(
                            xt_ps[:, j, :],
                            s["x"][:, jp * 128 : (jp + 1) * 128],
                            ident_sb[:],
                            is_transpose=True,
                            start=(j == 0),
                            stop=(j == 3),
                            skip_group_check=True,
                        )
                    xt_sb = xtpool.tile([128, 4, 128], F32R, tag="xt")
                    nc.any.tensor_copy(out=xt_sb[:], in_=xt_ps[:])
                    s["xt"][q] = xt_sb

                def stage_a(n, c, q):
                    s = chunk_state(n, c)
                    xt_sb = s["xt"].pop(q)
                    # 2 pair-matmuls into one 2-bank PSUM tile (each matmul
                    # stays inside its own 2KB bank), one batched drain
                    for h in range(2):
                        z_ps = ps_z.tile([C_IN, 2, 512], F32, tag="zp")
                        for jj in range(2):
                            nc.tensor.matmul(
                                z_ps[:, jj, 0 : 2 * K * V],
                                xt_sb[:, 2 * h + jj, :],
                                ma_sb[:],
                                start=True,
                                stop=True,
                            )
                        t0 = 8 * q + 4 * h
                        nc.any.tensor_copy(
                            out=s["z"][:, t0 : t0 + 4, :, :],
                            in_=z_ps[:, :, 0 : 2 * K * V],
                        )

                def stage_b(n, c, q):
                    s = chunk_state(n, c)
                    o_ps = ps_o.tile([C_OUT, 8, V], F32, tag="op")
                    for k in range(K):
                        nc.tensor.matmul(
                            o_ps[:],
                            wt_sb[:, k, :],
                            s["z"][:, 8 * q : 8 * (q + 1), k, :],
                            start=(k == 0),
                            stop=(k == K - 1),
                        )
                    nc.vector.tensor_add(
                        out=s["o"][:, 8 * q : 8 * (q + 1), :],
                        in0=o_ps[:],
                        in1=bias_sb[:],
                    )
                    if q == QG - 1:
                        # separate engine queue from the x-input DMAs so the
                        # in/out streams run on different DMA queues
                        nc.gpsimd.dma_start(
                            out=out[n, :, c * TC : (c + 1) * TC, :],
                            in_=s["o"][:],
                        )
                        del st[(n, c)]

                for i in range(len(groups) + 2):
                    if i < len(groups):
                        stage_tp(*groups[i])
                    if 1 <= i < len(groups) + 1:
                        stage_a(*groups[i - 1])
                    if i >= 2:
                        stage_b(*groups[i - 2])

    nc.compile()
    return nc


def prep_weights(A, W, b):
    A = np.asarray(A, np.float32)
    W = np.asarray(W, np.float32)
    b = np.asarray(b, np.float32)
    wt = np.ascontiguousarray(
        W.reshape(K, C_OUT, C_IN).transpose(2, 0, 1)
    )  # [ci, k, c]
    acat = np.ascontiguousarray(A.transpose(1, 0, 2))  # [v, k, w]
    ma = np.zeros((128, 2, K, V), np.float32)
    ma[0:64, 0] = acat
    ma[64:128, 1] = acat
    bias2 = np.einsum("kc,kw->cw", b.reshape(K, C_OUT), A.sum(axis=1))
    bias2r = np.ascontiguousarray(
        np.broadcast_to(bias2[:, None, :], (C_OUT, 8, V))
    ).astype(np.float32)
    ident = np.eye(128, dtype=np.float32)
    return wt, ma, bias2r, ident


_NC_CACHE = {}


def get_nc(reps: int = 1):
    if reps not in _NC_CACHE:
        _NC_CACHE[reps] = build(reps)
    return _NC_CACHE[reps]


def make_in_maps(x, A, W, b):
    x = np.asarray(x, np.float32)
    wt, ma, bias2r, ident = prep_weights(A, W, b)
    return [
        {
            "xs": np.ascontiguousarray(x[i * N_PER_CORE : (i + 1) * N_PER_CORE]),
            "wt": wt,
            "ma": ma,
            "bias2r": bias2r,
            "ident": ident,
        }
        for i in range(N_CORES)
    ]


def run(x, A, W, b, reps: int = 1):
    nc = get_nc(reps)
    in_maps = make_in_maps(x, A, W, b)
    res = run_bass_kernel_spmd(nc, in_maps, list(range(N_CORES)))
    return np.concatenate(
        [np.asarray(res.results[i]["out"]) for i in range(N_CORES)], axis=0
    )


def kernel(x, A, W, b):
    return run(x, A, W, b, reps=1)


# revision 9
# speedup vs baseline: 1.0195x; 1.0195x over previous
"""Trainium2 Bass kernel for ConvTemporalGraphical (gnn_message_passing).

Reference computation (fp32):
    y   = einsum('nctv,oc->notv', x, W) + b        # 1x1 conv channel mix
    out = einsum('nkctv,kvw->nctw', y.reshape(n,K,C,t,v), A)

Shapes: x [16,128,256,64] f32, A [3,64,64], W [384,128], b [384].

Strategy (8 NeuronCores, data-parallel over N, 2 samples per core):
  The two contractions are reordered as
      Z_k[ci,t,w] = sum_v x[ci,t,v] * A[k,v,w]          (graph mixing first)
      out[c,t,w]  = sum_k sum_ci W[(k,c),ci] * Z_k[ci,t,w] + bias2[c,w]
  with bias2[c,w] = sum_{k,v} b[(k,c)] A[k,v,w] (host-precomputed).

  On-device per (n, 32-t chunk):
    1. DMA x tile [ci=128, 32*64] (contiguous 8KB/partition).
    2. PE-transpose per 2-t pair: [ci,128] -> xt [(t0 v|t1 v), ci], fp32r.
       Four transposes share one PSUM tile so the drain is one [128,512] copy.
    3. Step A matmul (fp32r, FD=384): lhsT=xt pair, rhs=MA where MA [128,384]
       is block-diag([Acat, Acat]), Acat[v,(k w)]=A[k,v,w]. The zero blocks
       keep the two t's of a pair independent while using all 128 partitions.
       Two pair-outputs share one PSUM tile; Z accumulates into a
       [ci, 32, 3, 64] SBUF buffer.
    4. Step B matmul (fp32r, FD=512): per 8-t group, accumulate over k in
       PSUM: lhsT=Wt[:,k,:] ([ci,c]), rhs=Z[:, g8, k, :] (strided).
    5. Drain with fused bias add (DVE) -> out tile [c, 32, 64] -> DMA out.

  fp32r (tf32-like) matmuls run at 1 cycle/row for FD>=256 with ~1.6e-4
  relative rounding error; the PE rounds operands internally so DMA/copy
  producers don't need explicit rounding passes. Transposing in fp32r is
  exact w.r.t. the final result: it pre-rounds x exactly as step A would.

kernel(**inputs) shards on host, runs the SPMD program on cores 0-7, and
concatenates the per-core outputs.
"""

import numpy as np

import concourse.bass as bass
import concourse.mybir as mybir
from concourse import bacc
from concourse.bass_utils import run_bass_kernel_spmd
from concourse.tile import TileContext

F32 = mybir.dt.float32
F32R = mybir.dt.float32r

N, C_IN, C_OUT, K, T, V = 16, 128, 128, 3, 256, 64
N_CORES = 8
N_PER_CORE = N // N_CORES  # 2
TC = 32                    # t-chunk size
N_CHUNKS = T // TC         # 8
QG = TC // 8               # 4 quad-groups (8 t's = 4 pairs) per chunk


def build(reps: int = 1):
    nc = bacc.Bacc(
        "TRN2", target_bir_lowering=False, debug=False, num_devices=N_CORES
    )
    xs = nc.dram_tensor("xs", [N_PER_CORE, C_IN, T, V], F32, kind="ExternalInput")
    wt = nc.dram_tensor("wt", [C_IN, K, C_OUT], F32, kind="ExternalInput")
    ma = nc.dram_tensor("ma", [128, 2, K, V], F32, kind="ExternalInput")
    bias2r = nc.dram_tensor("bias2r", [C_OUT, 8, V], F32, kind="ExternalInput")
    ident = nc.dram_tensor("ident", [128, 128], F32, kind="ExternalInput")
    out = nc.dram_tensor(
        "out", [N_PER_CORE, C_OUT, T, V], F32, kind="ExternalOutput"
    )

    with TileContext(nc) as tc:
        with (
            tc.tile_pool(name="const", bufs=1) as cpool,
            tc.tile_pool(name="xin", bufs=8) as xpool,
            tc.tile_pool(name="xt", bufs=3) as xtpool,
            tc.tile_pool(name="z", bufs=2) as zpool,
            tc.tile_pool(name="o", bufs=3) as opool,
            tc.tile_pool(name="ps_xt", bufs=2, space="PSUM") as ps_xt,
            tc.tile_pool(name="ps_z", bufs=2, space="PSUM") as ps_z,
            tc.tile_pool(name="ps_o", bufs=2, space="PSUM") as ps_o,
        ):
            # consts on the gpsimd DMA queue so the sync queue's first x-tile
            # descriptor issues immediately
            ident_sb = cpool.tile([128, 128], F32R, tag="ident")
            nc.gpsimd.dma_start(out=ident_sb[:], in_=ident[:].bitcast(F32R))
            wt_sb = cpool.tile([C_IN, K, C_OUT], F32R, tag="wt")
            nc.gpsimd.dma_start(out=wt_sb[:], in_=wt[:].bitcast(F32R))
            ma_sb = cpool.tile([128, 2, K, V], F32R, tag="ma")
            nc.gpsimd.dma_start(out=ma_sb[:], in_=ma[:].bitcast(F32R))
            bias_sb = cpool.tile([C_OUT, 8, V], F32, tag="bias")
            nc.gpsimd.dma_start(out=bias_sb[:], in_=bias2r[:])

            # Software-pipelined emission: PE's stream is in-order, so a
            # matmul that depends on a same-stage drain stalls the PE for
            # the full DVE/ACT round trip. Emit transposes of group i,
            # step A of group i-1, and step B of group i-2 so every PE op's
            # producer drain has a full group-time to land.
            for _ in range(reps):
                groups = [
                    (n, c, q)
                    for n in range(N_PER_CORE)
                    for c in range(N_CHUNKS)
                    for q in range(QG)
                ]
                st = {}  # (n, c) -> chunk state

                def chunk_state(n, c):
                    if (n, c) not in st:
                        st[(n, c)] = {
                            "z": zpool.tile(
                                [C_IN, TC, K, V], F32R, tag="z", name="z_sb"
                            ),
                            "o": opool.tile(
                                [C_OUT, TC, V], F32, tag="o", name="o_sb"
                            ),
                            "xt": {},
                        }
                    return st[(n, c)]

                def stage_tp(n, c, q):
                    s = chunk_state(n, c)
                    # per-group x tile (8 t's): finer DMA granularity lets the
                    # first transposes start as soon as 256KB (not 1MB) landed
                    x_sb = xpool.tile([C_IN, 8 * V], F32R, tag="x", name="x_sb")
                    t0 = c * TC + 8 * q
                    nc.sync.dma_start(
                        out=x_sb[:],
                        in_=xs[n, :, t0 : t0 + 8, :].bitcast(F32R),
                    )
                    # 4 transposes -> one PSUM bank as ONE accumulation group
                    # (start clears the bank, so only the first sets it)
                    xt_ps = ps_xt.tile([128, 4, 128], F32R, tag="xtp")
                    for j in range(4):
                        nc.tensor.matmul(
                            xt_ps[:, j, :],
                            x_sb[:, j * 128 : (j + 1) * 128],
                            ident_sb[:],
                            is_transpose=True,
                            start=(j == 0),
                            stop=(j == 3),
                            skip_group_check=True,
                        )
                    xt_sb = xtpool.tile([128, 4, 128], F32R, tag="xt")
                    nc.any.tensor_copy(out=xt_sb[:], in_=xt_ps[:])
                    s["xt"][q] = xt_sb

                def stage_a(n, c, q):
                    s = chunk_state(n, c)
                    xt_sb = s["xt"].pop(q)
                    # 2 pair-matmuls into one 2-bank PSUM tile (each matmul
                    # stays inside its own 2KB bank), one batched drain
                    for h in range(2):
                        z_ps = ps_z.tile([C_IN, 2, 512], F32, tag="zp")
                        for jj in range(2):
                            nc.tensor.matmul(
                                z_ps[:, jj, 0 : 2 * K * V],
                                xt_sb[:, 2 * h + jj, :],
                                ma_sb[:],
                                start=True,
                                stop=True,
                            )
                        t0 = 8 * q + 4 * h
                        nc.any.tensor_copy(
                            out=s["z"][:, t0 : t0 + 4, :, :],
                            in_=z_ps[:, :, 0 : 2 * K * V],
                        )

                def stage_b(n, c, q):
                    s = chunk_state(n, c)
                    o_ps = ps_o.tile([C_OUT, 8, V], F32, tag="op")
                    for k in range(K):
                        nc.tensor.matmul(
                            o_ps[:],
                            wt_sb[:, k, :],
                            s["z"][:, 8 * q : 8 * (q + 1), k, :],
                            start=(k == 0),
                            stop=(k == K - 1),
                        )
                    nc.vector.tensor_add(
                        out=s["o"][:, 8 * q : 8 * (q + 1), :],
                        in0=o_ps[:],
                        in1=bias_sb[:],
                    )
                    if q == QG - 1:
                        # separate engine queue from the x-input DMAs so the
                        # in/out streams run on different DMA queues
                        nc.gpsimd.dma_start(
                            out=out[n, :, c * TC : (c + 1) * TC, :],
                            in_=s["o"][:],
                        )
                        del st[(n, c)]

                for i in range(len(groups) + 2):
                    if i < len(groups):
                        stage_tp(*groups[i])
                    if 1 <= i < len(groups) + 1:
                        stage_a(*groups[i - 1])
                    if i >= 2:
                        stage_b(*groups[i - 2])

    nc.compile()
    return nc


def prep_weights(A, W, b):
    A = np.asarray(A, np.float32)
    W = np.asarray(W, np.float32)
    b = np.asarray(b, np.float32)
    wt = np.ascontiguousarray(
        W.reshape(K, C_OUT, C_IN).transpose(2, 0, 1)
    )  # [ci, k, c]
    acat = np.ascontiguousarray(A.transpose(1, 0, 2))  # [v, k, w]
    ma = np.zeros((128, 2, K, V), np.float32)
    ma[0:64, 0] = acat
    ma[64:128, 1] = acat
    bias2 = np.einsum("kc,kw->cw", b.reshape(K, C_OUT), A.sum(axis=1))
    bias2r = np.ascontiguousarray(
        np.broadcast_to(bias2[:, None, :], (C_OUT, 8, V))
    ).astype(np.float32)
    ident = np.eye(128, dtype=np.float32)
    return wt, ma, bias2r, ident


_NC_CACHE = {}


def get_nc(reps: int = 1):
    if reps not in _NC_CACHE:
        _NC_CACHE[reps] = build(reps)
    return _NC_CACHE[reps]


def make_in_maps(x, A, W, b):
    x = np.asarray(x, np.float32)
    wt, ma, bias2r, ident = prep_weights(A, W, b)
    return [
        {
            "xs": np.ascontiguousarray(x[i * N_PER_CORE : (i + 1) * N_PER_CORE]),
            "wt": wt,
            "ma": ma,
            "bias2r": bias2r,
            "ident": ident,
        }
        for i in range(N_CORES)
    ]


def run(x, A, W, b, reps: int = 1):
    nc = get_nc(reps)
    in_maps = make_in_maps(x, A, W, b)
    res = run_bass_kernel_spmd(nc, in_maps, list(range(N_CORES)))
    return np.concatenate(
        [np.asarray(res.results[i]["out"]) for i in range(N_CORES)], axis=0
    )


def kernel(x, A, W, b):
    return run(x, A, W, b, reps=1)
